# revision 18
# baseline (speedup 1.0000x reference)
"""DANet (dual attention) Trainium2 kernel, v3.

Problem shapes (hardcoded): x [4, 64, 64, 64] f32, O = 16, N = H*W = 4096.
Sharding: 8 cores = 4 batches x 2 query-row halves (2048 query rows each).

v3 design notes (vs v2 baseline at ~218us/iter):
 - ACT is the pace engine (64 exps of [128,1024], ~1.0us each); everything
   else is kept off ACT: all PSUM drains moved to DVE, combine adds to
   GpSimd (Pool), channel-attention folded into the PE/DVE slack.
 - all PE matmuls are bf16 moving operands (except rbp: f32 rec moving):
   q+k are ONE merged projection (stationary wpack[:, :32] -> [32, N]),
   energy runs on bf16 xt, oc's moving operand is xf1h (bf16 x).
 - attv accumulates all 32 key-chunks of a query-half into one PSUM tile
   (start at i==0, stop at i==31): 8 burst drains -> 2 drains per body.
 - vt carries the denominator ones column FIRST (u=0) so the drained acc
   row 0 lands on partition 0, where reciprocal_approx_fast works.
 - the 2x residual rides the oc drain: ocsb2 = 2*xq1 + oc via one DVE
   scalar_tensor_tensor per [64,512] piece; final ob = t1+ocsb2 on GpSimd.
 - PSUM: st 2x[128,1024] (4 banks) + acc 1x[65,1024] (2) + gen 2x1 bank
   (qk/v/en/at/oc/rbp time-share) = 8 banks exactly.
"""

import sys

for _p in ("/opt/trn_rl_repo",):
    if _p not in sys.path:
        sys.path.append(_p)

import numpy as np
import ml_dtypes
from contextlib import ExitStack

import concourse.bass as bass
import concourse.bacc as bacc
import concourse.mybir as mybir
import concourse.tile as tile

F32 = mybir.dt.float32
BF16 = mybir.dt.bfloat16
AF = mybir.ActivationFunctionType
AX = mybir.AxisListType
ALU = mybir.AluOpType

B, C, H, W = 4, 64, 64, 64
N = H * W          # 4096
O = C // 4         # 16
NQ = N // 2        # 2048 query rows per core
NK_CH = N // 128   # 32 key chunks of 128
U = C + 1          # 65: ones column + 64 v channels


def build_program(gamma_ca: float = 0.0, repeat: int = 1, loop_n: int = 0,
                  debug_dump: bool = False):
    nc = bacc.Bacc(
        "TRN2", target_bir_lowering=False, debug=False, num_devices=8
    )

    xf1h_d = nc.dram_tensor("xf1h", [U, N], BF16, kind="ExternalInput").ap()
    xq1_d = nc.dram_tensor("xq1", [C, NQ], F32, kind="ExternalInput").ap()
    xt_d = nc.dram_tensor("xt", [128, NK_CH * C], BF16,
                          kind="ExternalInput").ap()
    # wpack cols: 0:16 q, 16:32 zero pad, 32:48 k, 48:112 g_sa*v
    # (the pad keeps k at out-partition 32, a legal engine start partition)
    wpack_d = nc.dram_tensor("wpack", [U, 3 * O + C], BF16,
                             kind="ExternalInput").ap()
    eyeg_d = nc.dram_tensor("eyeg", [C, C], F32, kind="ExternalInput").ap()
    out_d = nc.dram_tensor("out", [C, NQ], F32, kind="ExternalOutput").ap()

    with tile.TileContext(nc) as tc:
        with ExitStack() as ctx:
            consts = ctx.enter_context(tc.tile_pool(name="consts", bufs=2))
            expp = ctx.enter_context(tc.tile_pool(name="expp", bufs=4))
            sm = ctx.enter_context(tc.tile_pool(name="sm", bufs=2))
            outp = ctx.enter_context(tc.tile_pool(name="outp", bufs=2))
            pst = ctx.enter_context(
                tc.tile_pool(name="pst", bufs=2, space="PSUM"))
            pacc = ctx.enter_context(
                tc.tile_pool(name="pacc", bufs=1, space="PSUM"))
            pgen = ctx.enter_context(
                tc.tile_pool(name="pgen", bufs=2, space="PSUM"))

            warm = sm.tile([1, 16], F32, tag="warm")
            nc.vector.memset(warm[:], 0.0)
            nc.scalar.activation(warm[:], warm[:], AF.Exp)

            def emit_all():
                for _rep in range(repeat):
                    _emit_body(tc, consts, expp, sm, outp, pst, pacc, pgen,
                               xf1h_d, xq1_d, xt_d, wpack_d, eyeg_d, out_d)

            if loop_n:
                with tc.For_i(
                    0, loop_n, 1,
                    hint_engines=(mybir.EngineType.PE,
                                  mybir.EngineType.Activation),
                ):
                    emit_all()
            else:
                emit_all()
    nc.compile()
    return nc


def _emit_body(tc, consts, expp, sm, outp, pst, pacc, pgen,
               xf1h_d, xq1_d, xt_d, wpack_d, eyeg_d, out_d):
    nc = tc.nc

    # ---- input DMAs ----
    wpack = consts.tile([U, 3 * O + C], BF16, tag="wpack")
    nc.sync.dma_start(wpack[:], wpack_d[:])
    xf1h = consts.tile([U, N], BF16, tag="xf1h")
    for j in range(4):
        nc.sync.dma_start(
            xf1h[:, j * 1024:(j + 1) * 1024],
            xf1h_d[:, j * 1024:(j + 1) * 1024])
    xt = consts.tile([128, NK_CH * C], BF16, tag="xt")
    nc.sync.dma_start(xt[:], xt_d[:])
    xq1 = consts.tile([C, NQ], F32, tag="xq1")
    nc.sync.dma_start(xq1[:], xq1_d[:])
    eyeg = consts.tile([C, C], F32, tag="eyeg")
    nc.sync.dma_start(eyeg[:], eyeg_d[:])

    ones1 = consts.tile([1, C], F32, tag="ones1")
    nc.vector.memset(ones1[:], 1.0)

    # vt: 64 v channels, then the ones column (u=64) for the denominator
    vt = consts.tile([128, NK_CH * U], BF16, tag="vt")
    vt3 = vt[:].rearrange("p (c u) -> p c u", u=U)
    nc.vector.memset(vt3[:, :, C:U], 1.0)

    # ---- state ----
    q_sb = consts.tile([O, NQ], BF16, tag="q")
    k_sb = consts.tile([O, N], BF16, tag="k")
    accsb = sm.tile([C, NQ], F32, tag="accsb")
    ocsb2 = sm.tile([C, NQ], F32, tag="ocsb2")
    attF = sm.tile([C, C], BF16, tag="attF")
    ex_tiles = {}
    accd = {}
    chan = {}
    qk_ps = {}

    def emit_qk_mm(j):
        pq = pgen.tile([3 * O, 512], F32, tag="gen", name=f"pqk{j}")
        nc.tensor.matmul(pq[:], wpack[:, 0:3 * O],
                         xf1h[:, j * 512:(j + 1) * 512],
                         start=True, stop=True)
        qk_ps[j] = pq

    def emit_qk_drain(j, k_on_act=False):
        pq = qk_ps.pop(j)
        if k_on_act:  # ACT is exp-idle in the prologue; split the drain
            nc.scalar.copy(k_sb[:, j * 512:(j + 1) * 512],
                           pq[2 * O:3 * O, :])
        else:
            nc.vector.tensor_copy(k_sb[:, j * 512:(j + 1) * 512],
                                  pq[2 * O:3 * O, :])
        if j < 4:  # q only spans the first NQ columns
            nc.vector.tensor_copy(q_sb[:, j * 512:(j + 1) * 512],
                                  pq[0:O, :])

    def emit_v_group(g):
        pv = pgen.tile([128, 4 * C], F32, tag="gen", name=f"pv{g}")
        for q4 in range(4):
            i = g * 4 + q4
            nc.tensor.matmul(pv[:, q4 * C:(q4 + 1) * C],
                             xf1h[:, i * 128:(i + 1) * 128],
                             wpack[:, 3 * O:3 * O + C],
                             start=True, stop=True)
        dst = vt3[:, g * 4:(g + 1) * 4, 0:C]
        src = pv[:].rearrange("p (c f) -> p c f", f=C)
        nc.vector.tensor_copy(dst, src)

    def emit_energy_burst(k):
        if k == 0:
            chan["en"] = pgen.tile([C, C], F32, tag="gen", name="en")
        en = chan["en"]
        for i in range(k * 8, k * 8 + 8):
            nc.tensor.matmul(en[:], xt[:, i * C:(i + 1) * C],
                             xt[:, i * C:(i + 1) * C],
                             start=(i == 0), stop=(i == NK_CH - 1))

    def emit_channel_softmax():
        en = chan.pop("en")
        emin = sm.tile([C, 1], F32, tag="emin")
        nc.vector.tensor_reduce(emin[:], en[:], axis=AX.X, op=ALU.min)
        ae = sm.tile([C, C], F32, tag="ae")
        esum = sm.tile([C, 1], F32, tag="esum")
        nc.scalar.activation(ae[:], en[:], AF.Exp, bias=emin[:], scale=-1.0,
                             accum_out=esum[:])
        esr = sm.tile([C, 1], F32, tag="esr")
        nc.vector.reciprocal(esr[:], esum[:])
        ac = sm.tile([C, C], F32, tag="ac")
        nc.vector.tensor_scalar_mul(ac[:], ae[:], esr[:])
        chan["ac"] = ac

    def emit_at():
        ac = chan.pop("ac")
        at = pgen.tile([C, C], F32, tag="gen", name="at")
        nc.tensor.matmul(at[:], ac[:], eyeg[:], start=True, stop=True)
        nc.vector.tensor_copy(attF[:], at[:])

    def emit_oc(h2):
        po = pgen.tile([C, 512], F32, tag="gen", name=f"oc{h2}")
        nc.tensor.matmul(po[:], attF[:],
                         xf1h[0:C, h2 * 512:(h2 + 1) * 512],
                         start=True, stop=True)
        # fold the 2x residual into the drain: ocsb2 = 2*xq1 + oc
        nc.vector.scalar_tensor_tensor(
            ocsb2[:, h2 * 512:(h2 + 1) * 512],
            xq1[:, h2 * 512:(h2 + 1) * 512], 2.0, po[:],
            ALU.mult, ALU.add)

    def emit_st(idx):
        p, i = idx // NK_CH, idx % NK_CH
        st = pst.tile([128, 1024], F32, tag="st", name=f"st{p}_{i}")
        for j2 in range(2):
            nc.tensor.matmul(
                st[:, j2 * 512:(j2 + 1) * 512],
                k_sb[:, i * 128:(i + 1) * 128],
                q_sb[:, p * 1024 + j2 * 512:p * 1024 + (j2 + 1) * 512],
                start=True, stop=True)
        ex = expp.tile([128, 1024], BF16, tag="ex", name=f"ex{p}_{i}")
        nc.scalar.activation(ex[:], st[:], AF.Exp)
        ex_tiles[idx] = ex

    def emit_attv(idx):
        p, i = idx // NK_CH, idx % NK_CH
        ex = ex_tiles.pop(idx)
        if i == 0:
            accd["t"] = pacc.tile([U, 1024], F32, tag="acc", name=f"acc{p}")
        for j2 in range(2):
            nc.tensor.matmul(accd["t"][:, j2 * 512:(j2 + 1) * 512],
                             vt[:, i * U:(i + 1) * U],
                             ex[:, j2 * 512:(j2 + 1) * 512],
                             start=(i == 0), stop=(i == NK_CH - 1))

    def emit_combine(p):
        ps = slice(p * 1024, (p + 1) * 1024)
        a = accd.pop("t")
        # denominator row first (partition 64 -> partition 0; the custom-DVE
        # reciprocal only works at partition offset 0), so rec/rbp overlap
        # the big accsb drain. For the final half (the kernel tail) the
        # accsb drain goes to ACT, which is exp-idle by then.
        den = sm.tile([1, 1024], F32, tag="den", name=f"den{p}", bufs=2)
        nc.vector.tensor_copy(den[:], a[C:U, :])
        rec = sm.tile([1, 1024], F32, tag="rec", name=f"rec{p}", bufs=2)
        nc.vector.reciprocal_approx_fast(rec[:], den[:])
        if p == 1:
            nc.scalar.copy(accsb[0:C, ps], a[0:C, :])
        else:
            nc.vector.tensor_copy(accsb[0:C, ps], a[0:C, :])
        for j2 in range(2):
            hs = slice(j2 * 512, (j2 + 1) * 512)
            gs = slice(p * 1024 + j2 * 512, p * 1024 + (j2 + 1) * 512)
            rbp = pgen.tile([C, 512], F32, tag="gen", name=f"rbp{p}{j2}")
            nc.tensor.matmul(rbp[:], ones1[:], rec[:, hs],
                             start=True, stop=True)
            t1 = sm.tile([C, 512], F32, tag="t1", name=f"t1{p}{j2}", bufs=2)
            nc.vector.tensor_mul(t1[:], accsb[0:C, gs], rbp[:])
            ob = outp.tile([C, 512], F32, tag="ob", name=f"ob{p}{j2}")
            nc.gpsimd.tensor_add(ob[:], t1[:], ocsb2[:, gs])
            nc.sync.dma_start(out_d[:, gs], ob[:])

    # ---- prologue: get the first exp going as early as possible ----
    emit_qk_mm(0)
    emit_qk_mm(1)
    emit_qk_drain(0, k_on_act=True)
    emit_qk_drain(1, k_on_act=True)

    # ---- main stream: skew-2 software pipeline over 64 chunks ----
    def qk_step(j):
        def f():
            emit_qk_mm(j)
            emit_qk_drain(j)
        return f

    weave = {
        0: [qk_step(4)],
        2: [lambda: emit_v_group(2), qk_step(5)],
        4: [lambda: emit_energy_burst(0)],
        5: [lambda: emit_energy_burst(1)],
        6: [lambda: emit_v_group(3), qk_step(6)],
        7: [lambda: emit_energy_burst(2)],
        8: [lambda: emit_energy_burst(3), qk_step(7)],
        9: [emit_channel_softmax],
        10: [lambda: emit_v_group(4), emit_at],
        11: [lambda: emit_oc(0)],
        13: [lambda: emit_oc(1)],
        14: [lambda: emit_v_group(5)],
        15: [lambda: emit_oc(2)],
        17: [lambda: emit_oc(3)],
        18: [lambda: emit_v_group(6)],
        22: [lambda: emit_v_group(7)],
    }

    emit_st(0)
    emit_st(1)
    emit_v_group(0)
    emit_v_group(1)
    emit_qk_mm(2)
    emit_qk_drain(2)
    emit_qk_mm(3)
    emit_qk_drain(3)
    for idx in range(2 * NK_CH):
        if idx + 2 < 2 * NK_CH:
            emit_st(idx + 2)
        emit_attv(idx)
        for fn in weave.get(idx, ()):
            fn()
        if idx % NK_CH == NK_CH - 1:
            emit_combine(idx // NK_CH)


# ---------------- host side ----------------

_PROGRAM_CACHE = {}


def _get_program(gamma_ca: float = 0.0):
    # gamma_ca rides in via the eyeg input; one program serves all.
    if "p" not in _PROGRAM_CACHE:
        _PROGRAM_CACHE["p"] = build_program()
    return _PROGRAM_CACHE["p"]


def build_in_maps(x, wq, bq, wk, bk, wv, bv, gamma_ca, gamma_sa):
    bf16 = np.dtype(ml_dtypes.bfloat16)
    x = np.asarray(x, dtype=np.float32)
    wq = np.asarray(wq, dtype=np.float32)
    bq = np.asarray(bq, dtype=np.float32)
    wk = np.asarray(wk, dtype=np.float32)
    bk = np.asarray(bk, dtype=np.float32)
    wv = np.asarray(wv, dtype=np.float32)
    bv = np.asarray(bv, dtype=np.float32)
    g_ca = float(np.asarray(gamma_ca).reshape(-1)[0])
    g_sa = float(np.asarray(gamma_sa).reshape(-1)[0])

    xf = x.reshape(B, C, N)
    xt_pre = [
        np.ascontiguousarray(
            xf[b].T.reshape(NK_CH, 128, C).transpose(1, 0, 2).reshape(
                128, NK_CH * C)).astype(bf16)
        for b in range(B)
    ]
    ones_row = np.ones((1, N), np.float32)
    qT1 = np.concatenate([wq.T, bq[None, :]], axis=0)
    kT1 = np.concatenate([wk.T, bk[None, :]], axis=0)
    wvT1 = g_sa * np.concatenate([wv.T, bv[None, :]], axis=0)
    pad = np.zeros((C + 1, O), np.float32)
    wpack = np.ascontiguousarray(
        np.concatenate([qT1, pad, kT1, wvT1], axis=1).astype(bf16))
    eyeg = np.ascontiguousarray(g_ca * np.eye(C, dtype=np.float32))

    in_maps = []
    for core in range(8):
        b, h = core // 2, core % 2
        # rotate: this core's query half first (softmax over keys is
        # permutation-invariant; only query column order matters)
        xrot = np.concatenate(
            [xf[b][:, h * NQ:(h + 1) * NQ], xf[b][:, (1 - h) * NQ:(2 - h) * NQ]],
            axis=1)
        xf1 = np.concatenate([xrot, ones_row], axis=0)
        in_maps.append({
            "xf1h": np.ascontiguousarray(xf1.astype(bf16)),
            "xq1": np.ascontiguousarray(xrot[:, 0:NQ]),
            "xt": xt_pre[b],
            "wpack": wpack,
            "eyeg": eyeg,
        })
    return in_maps


LAST_RESULTS = None


def kernel(x, wq, bq, wk, bk, wv, bv, gamma_ca, gamma_sa):
    global LAST_RESULTS
    from concourse.bass_utils import run_bass_kernel_spmd

    nc = _get_program()
    in_maps = build_in_maps(x, wq, bq, wk, bk, wv, bv, gamma_ca, gamma_sa)

    res = run_bass_kernel_spmd(nc, in_maps, list(range(8)))
    LAST_RESULTS = res
    out = np.empty((B, C, N), np.float32)
    for core in range(8):
        b, h = core // 2, core % 2
        out[b, :, h * NQ:(h + 1) * NQ] = res.results[core]["out"]
    return out.reshape(B, C, H, W)
